# revision 1
# baseline (speedup 1.0000x reference)
"""AFT (Attention-Free Transformer) encoder block on 8 TRN2 NeuronCores.

Strategy
--------
Two SPMD launches:

Phase 1 (sequence-sharded): each core takes a T/8 slice of the sequence
axis for ALL batches, computes K = LN1(x) @ Wk for its slice and reduces
max over the batch axis locally -> M0 slice [T/8, D].  The host merely
concatenates the 8 slices (pure gather).

Phase 2 (batch-sharded): each core owns one batch element and computes the
whole block.  The batch-max M0 (replicated input) makes exp_K local.

Math notes:
 - exp_w's row-max stabilization cancels exactly in num/den (per-row
   factor), so we use exp(w) directly -> no row-max, no extra pass.
 - bk cancels between K and max_b(K) (max is shift-equivariant), so the
   K-projection bias is skipped in both phases (exact when bk == 0).
 - rsqrt = exp(-0.5*ln(x)) and sigmoid = 1/(1+exp(-x)): keeps every ACT
   call in the {Ln, Exp} table set (a set switch costs ~2.7us); only the
   FFN's Gelu needs one switch.

Matmul dtype is float32r (= tf32, full PE rate at moving dim >= 256;
LDWEIGHTS ~188 ns hides under N=512 moving ~213 ns).  f32r operand tiles
are rounded by their producing ACT/DVE write; DMA'd weights are raw-bit
loads.  Transposes stay plain fp32 (f32r transpose fails walrus codegen).

Per-core layout (phase 2), tiles [128, *] fp32 unless noted:
  hT   = LN1(x)^T               4x[128,2048] f32r (PE transpose of h)
  E    = exp(K - M0)           16x[128, 512] f32r (natural [t,d] rows)
  U    = E * (h@Wv + bv)       16x[128, 512] f32r
  G    = sigmoid(h@Wq + bq)    16x[128, 512] (natural)
  EW   = exp(w)^T streamed      [2048s, 128t] cols f32r (transp + exp)
  num/den: lhsT=EW col-block (stationary), rhs=U/E (moving N=512),
           accumulated over s in two PSUM banks per t-chunk
  Yt2  = G * num/den -> transpose -> Yt2T 4x[128,2048] f32r (hT slots)
  attn = Yt2T^T @ Wo + bo + x   natural (lhsT=Yt2T slices, rhs=Wo)
  LN2 -> h2 -> transpose -> h2T (reuses slots); FFN strips of 512:
  g1T[h,512] = gelu(W1^T h2T + b1) f32r; out += gelu(g1T^T W2 + b2).
"""

import sys

for _p in ("/opt/trn_rl_repo",):
    if _p not in sys.path:
        sys.path.insert(0, _p)

import numpy as np

import concourse.bass as bass
import concourse.bacc as bacc
import concourse.tile as tile
from concourse import mybir
from concourse import bass_utils
from concourse.masks import make_identity

B, T, D, H = 8, 2048, 512, 2048
EPS = 1e-5
NCORES = 8
P = 128
TS = T // NCORES          # seq rows per core in phase 1
NT = T // P               # 16 row tiles of the full sequence
ND = D // P               # 4 d-chunks
NH = H // P               # 16 h-chunks
F32 = mybir.dt.float32
F32R = mybir.dt.float32r
AF = mybir.ActivationFunctionType
ALU = mybir.AluOpType
PSUM = bass.MemorySpace.PSUM

TRACE = False             # test harness sets True to capture NTFF profiles
LAST_RESULTS = []         # BassKernelResults per phase from the last kernel()


def _ln_stats(nc, pool, x_tile, mvall, col):
    """bn stats for one [P, D] tile; mean/var into mvall[:, 2c:2c+2]."""
    stats = pool.tile([P, 6], F32, tag="ln_stats")
    nc.vector.bn_stats(out=stats, in_=x_tile)
    nc.vector.bn_aggr(out=mvall[:, 2 * col:2 * col + 2], in_=stats)


def _ln_rstd_batch(nc, pool, mvall, n, eps_tile, tag, col0=0):
    """rstd for n tiles at once: 1/ACT-sqrt(var+eps) on [P, n].

    Batching keeps ACT table switches (~2.7us each) rare: consecutive
    Sqrt calls share the loaded set, so grouped call sites cost one
    switch total as long as no other transcendental interleaves.
    """
    var_view = mvall.rearrange("p (n two) -> p n two", two=2)[:, col0:col0 + n, 1]
    rstd = pool.tile([P, n], F32, tag=f"{tag}_rstd")
    nc.scalar.activation(out=rstd, in_=var_view, func=AF.Sqrt, bias=eps_tile)
    nc.vector.reciprocal(out=rstd, in_=rstd)
    return rstd


def _ln_apply(nc, pool, x_tile, mvall, col, rstd_all, g_bc, b_bc,
              gp_apply=False, rcol=None):
    """Apply LayerNorm to one tile given batched stats/rstd."""
    rcol = col if rcol is None else rcol
    h_tile = pool.tile([P, D], F32, tag="ln_h")
    nc.vector.tensor_scalar(
        out=h_tile, in0=x_tile,
        scalar1=mvall[:, 2 * col:2 * col + 1],
        scalar2=rstd_all[:, rcol:rcol + 1],
        op0=ALU.subtract, op1=ALU.mult)
    eng = nc.gpsimd if gp_apply else nc.vector
    eng.tensor_tensor(out=h_tile, in0=h_tile, in1=g_bc, op=ALU.mult)
    eng.tensor_tensor(out=h_tile, in0=h_tile, in1=b_bc, op=ALU.add)
    return h_tile


def _transpose_tile(nc, psum_pool, dst, dst_col, src, identity, copy_eng=None):
    """dst[:, dst_col:dst_col+P] = src[:, :P].T via PE transpose (fp32)."""
    pt = psum_pool.tile([P, P], F32, tag="tpsum")
    nc.tensor.transpose(pt, src, identity)
    if copy_eng is None:
        nc.vector.tensor_copy(out=dst[:, dst_col:dst_col + P], in_=pt)
    else:
        copy_eng.activation(out=dst[:, dst_col:dst_col + P], in_=pt,
                            func=AF.Copy)


def _load_rows(nc, pool, dram_ap, n_tiles, tag, width, dtype=F32):
    """Load [P, width] row tiles of a DRAM matrix into a list of tiles.

    dtype=F32R does a raw-bits DMA into an f32r-typed tile (the PE reads
    tf32 precision either way; the verifier accepts DMA producers).
    """
    out = []
    for j in range(n_tiles):
        t = pool.tile([P, width], dtype, tag=f"{tag}{j}")
        src = dram_ap[j * P:(j + 1) * P, :]
        if dtype is F32R:
            src = src.bitcast(F32R)
        nc.sync.dma_start(out=t, in_=src)
        out.append(t)
    return out


def _bc(nc, pool, dram_ap, tag):
    """Broadcast a [D] vector across 128 partitions."""
    t = pool.tile([P, D], F32, tag=tag)
    nc.gpsimd.dma_start(out=t, in_=dram_ap.partition_broadcast(P))
    return t


def _part_bias(nc, pool, dram_ap, n, tag):
    """Load a [n*P] vector as per-partition bias columns [P, n]."""
    t = pool.tile([P, n], F32, tag=tag)
    for k in range(n):
        nc.sync.dma_start(
            out=t[:, k:k + 1],
            in_=dram_ap[k * P:(k + 1) * P].rearrange("(p o) -> p o", o=1))
    return t


def _build_phase1():
    """Per core: rows = [B, TS] b-major flattened; out M0 = max_b (LN1(x)@Wk)."""
    nc = bacc.Bacc(trn_type="TRN2", target_bir_lowering=False, debug=False,
                   num_devices=NCORES)
    xs = nc.dram_tensor("xs", [B * TS, D], F32, kind="ExternalInput").ap()
    g1 = nc.dram_tensor("ln1_g", [D], F32, kind="ExternalInput").ap()
    b1 = nc.dram_tensor("ln1_b", [D], F32, kind="ExternalInput").ap()
    wk = nc.dram_tensor("Wk", [D, D], F32, kind="ExternalInput").ap()
    m0 = nc.dram_tensor("M0", [TS, D], F32, kind="ExternalOutput").ap()

    n_tiles = B * TS // P          # 16
    tiles_per_b = TS // P          # 2

    with tile.TileContext(nc) as tc:
        pools = []

        def alloc(**kw):
            p = tc.alloc_tile_pool(**kw)
            pools.append(p)
            return p

        pc = alloc(name="consts", bufs=1)
        pwk = alloc(name="wk", bufs=1)
        pk = alloc(name="ks", bufs=1)
        px = alloc(name="xrows", bufs=1)
        ps = alloc(name="stream", bufs=3)
        ppt = alloc(name="psum_t", bufs=2, space=PSUM)
        ppm = alloc(name="psum_mm", bufs=2, space=PSUM)

        identity = pc.tile([P, P], F32)
        make_identity(nc, identity)
        eps_tile = pc.tile([P, 1], F32)
        nc.vector.memset(eps_tile, EPS)
        g_bc = _bc(nc, pc, g1, "g_bc")
        b_bc = _bc(nc, pc, b1, "b_bc")
        wk_sb = _load_rows(nc, pwk, wk, ND, "wk", D, dtype=F32R)

        x_sb = []
        mvall = pc.tile([P, 2 * n_tiles], F32)
        for j in range(n_tiles):
            x_tile = px.tile([P, D], F32, tag=f"x{j}", name=f"x{j}")
            nc.sync.dma_start(out=x_tile, in_=xs[j * P:(j + 1) * P, :])
            _ln_stats(nc, ps, x_tile, mvall, j)
            x_sb.append(x_tile)
        rstd_all = _ln_rstd_batch(nc, pc, mvall, n_tiles, eps_tile, "r1")

        k_sb = []
        for j in range(n_tiles):
            h_tile = _ln_apply(nc, ps, x_sb[j], mvall, j, rstd_all,
                               g_bc, b_bc, gp_apply=True)
            hT = ps.tile([P, P * ND], F32R, tag="hT")
            for dj in range(ND):
                _transpose_tile(nc, ppt, hT, dj * P,
                                h_tile[:, dj * P:(dj + 1) * P], identity)
            pk_t = ppm.tile([P, D], F32, tag="kpsum")
            for dj in range(ND):
                nc.tensor.matmul(
                    pk_t, hT[:, dj * P:(dj + 1) * P], wk_sb[dj],
                    start=(dj == 0), stop=(dj == ND - 1))
            kt = pk.tile([P, D], F32, tag=f"k{j}")
            nc.scalar.activation(out=kt, in_=pk_t, func=AF.Copy)
            k_sb.append(kt)

        for half in range(tiles_per_b):
            acc = ps.tile([P, D], F32, tag="macc")
            nc.vector.tensor_tensor(
                out=acc, in0=k_sb[half], in1=k_sb[tiles_per_b + half],
                op=ALU.max)
            for b in range(2, B):
                nc.vector.tensor_tensor(
                    out=acc, in0=acc, in1=k_sb[b * tiles_per_b + half],
                    op=ALU.max)
            nc.sync.dma_start(out=m0[half * P:(half + 1) * P, :], in_=acc)

        for p in reversed(pools):
            p.release()

    nc.compile()
    return nc


def _build_phase2():
    nc = bacc.Bacc(trn_type="TRN2", target_bir_lowering=False, debug=False,
                   num_devices=NCORES)
    ap = {}
    ap["x"] = nc.dram_tensor("x", [T, D], F32, kind="ExternalInput").ap()
    ap["M0"] = nc.dram_tensor("M0", [T, D], F32, kind="ExternalInput").ap()
    ap["w"] = nc.dram_tensor("w", [T, T], F32, kind="ExternalInput").ap()
    for n, shp in [("ln1_g", [D]), ("ln1_b", [D]), ("Wk", [D, D]),
                   ("Wv", [D, D]), ("bv", [D]), ("Wq", [D, D]), ("bq", [D]),
                   ("Wo", [D, D]), ("bo", [D]), ("ln2_g", [D]), ("ln2_b", [D]),
                   ("W1", [D, H]), ("b1", [H]), ("W2", [H, D]), ("b2", [D])]:
        ap[n] = nc.dram_tensor(n, shp, F32, kind="ExternalInput").ap()
    out_d = nc.dram_tensor("out", [T, D], F32, kind="ExternalOutput").ap()

    with tile.TileContext(nc) as tc:
        # SBUF pool stack (LIFO release):
        #   pc | ptm (hT -> Yt2T -> h2T) | pe | pu | pg | pw | ps_a
        #   ... A ... pop ps_a, pw
        #   push pew, pbs ... B ... pop pbs, pew, pg, pu, pe
        #   push pout, pwo, ps_c ... C ... pop ps_c, pwo
        #   push pfw, pg1, pds ... D ... pop all
        pc = tc.alloc_tile_pool(name="consts", bufs=1)
        ptm = tc.alloc_tile_pool(name="tmat", bufs=1)
        pe = tc.alloc_tile_pool(name="rows_e", bufs=1)
        pu = tc.alloc_tile_pool(name="rows_u", bufs=1)
        pg = tc.alloc_tile_pool(name="gate", bufs=1)
        pw = tc.alloc_tile_pool(name="wproj", bufs=1)
        px = tc.alloc_tile_pool(name="xrows", bufs=1)
        ps_a = tc.alloc_tile_pool(name="stream_a", bufs=2)
        ppt = tc.alloc_tile_pool(name="psum_t", bufs=2, space=PSUM)
        ppm = tc.alloc_tile_pool(name="psum_mm", bufs=2, space=PSUM)

        identity = pc.tile([P, P], F32)
        make_identity(nc, identity)
        identity_bf = pc.tile([P, P], mybir.dt.bfloat16)
        make_identity(nc, identity_bf)
        eps_tile = pc.tile([P, 1], F32)
        nc.vector.memset(eps_tile, EPS)

        # ---------------- Stage A: LN1, hT, E, U, G --------------------
        g1_bc = _bc(nc, pw, ap["ln1_g"], "g1_bc")
        b1g_bc = _bc(nc, pw, ap["ln1_b"], "b1g_bc")
        bv_bc = _bc(nc, pw, ap["bv"], "bv_bc")
        bq_bc = _bc(nc, pw, ap["bq"], "bq_bc")
        wk_sb = _load_rows(nc, pw, ap["Wk"], ND, "wk", D, dtype=F32R)
        wv_sb = _load_rows(nc, pw, ap["Wv"], ND, "wv", D, dtype=F32R)
        wq_sb = _load_rows(nc, pw, ap["Wq"], ND, "wq", D, dtype=F32R)

        hT = [ptm.tile([P, T], F32R, tag=f"tm{dj}", name=f"hT{dj}")
              for dj in range(ND)]
        # batched LN1 statistics: all tiles' stats first, one Ln+Exp pair
        x_sb = []
        mvall = pc.tile([P, 2 * NT], F32, name="mvall1")
        for j in range(NT):
            x_tile = px.tile([P, D], F32, tag=f"x{j}", name=f"x{j}")
            nc.sync.dma_start(out=x_tile, in_=ap["x"][j * P:(j + 1) * P, :])
            _ln_stats(nc, ps_a, x_tile, mvall, j)
            x_sb.append(x_tile)
        rstd_all = _ln_rstd_batch(nc, pc, mvall, NT, eps_tile, "r1")

        e_sb, u_sb, g_sb = [], [], []
        for j in range(NT):
            h_tile = _ln_apply(nc, ps_a, x_sb[j], mvall, j, rstd_all,
                               g1_bc, b1g_bc, gp_apply=True)
            for dj in range(ND):
                _transpose_tile(nc, ppt, hT[dj], j * P,
                                h_tile[:, dj * P:(dj + 1) * P], identity)
            hT_blk = [hT[dj][:, j * P:(j + 1) * P] for dj in range(ND)]
            # K (no bias; cancels with M0) -> E = exp(K - M0)
            pk_t = ppm.tile([P, D], F32, tag="kv_psum")
            for dj in range(ND):
                nc.tensor.matmul(pk_t, hT_blk[dj], wk_sb[dj],
                                 start=(dj == 0), stop=(dj == ND - 1))
            m_tile = ps_a.tile([P, D], F32, tag="m0")
            nc.sync.dma_start(out=m_tile, in_=ap["M0"][j * P:(j + 1) * P, :])
            nc.vector.tensor_sub(out=pk_t, in0=pk_t, in1=m_tile)
            et = pe.tile([P, D], F32R, tag=f"e{j}")
            nc.scalar.activation(out=et, in_=pk_t, func=AF.Exp)
            e_sb.append(et)
            # V then U = E * (V + bv)
            pv_t = ppm.tile([P, D], F32, tag="kv_psum")
            for dj in range(ND):
                nc.tensor.matmul(pv_t, hT_blk[dj], wv_sb[dj],
                                 start=(dj == 0), stop=(dj == ND - 1))
            nc.vector.tensor_add(out=pv_t, in0=pv_t, in1=bv_bc)
            ut = pu.tile([P, D], F32R, tag=f"u{j}")
            nc.vector.tensor_mul(out=ut, in0=et.bitcast(F32), in1=pv_t)
            u_sb.append(ut)
            # G = sigmoid(Q + bq) = 1 / (1 + exp(-(Q + bq)))
            pq_t = ppm.tile([P, D], F32, tag="kv_psum")
            for dj in range(ND):
                nc.tensor.matmul(pq_t, hT_blk[dj], wq_sb[dj],
                                 start=(dj == 0), stop=(dj == ND - 1))
            nc.vector.tensor_add(out=pq_t, in0=pq_t, in1=bq_bc)
            gt = pg.tile([P, D], F32, tag=f"g{j}")
            nc.scalar.activation(out=gt, in_=pq_t, func=AF.Exp, scale=-1.0)
            nc.vector.tensor_scalar_add(out=gt, in0=gt, scalar1=1.0)
            nc.vector.reciprocal_approx_fast(out=gt, in_=gt)
            g_sb.append(gt)
        ps_a.release()
        px.release()
        pw.release()
        ppm.release()

        # ---------- Stage B: einsum + gate -----------------------------
        # num[t,d] = sum_s exp(w[t,s]) * U[s,d]; den likewise with E.
        # lhsT = EW column block [128s, 128t] (stationary), rhs = U/E rows
        # (moving N=512); accumulate over the 16 s-blocks in PSUM.
        # EW path runs in bf16 (|w| < 0.04, so the cast costs ~1e-4 rel):
        # bf16 PE transposes are 4x cheaper than fp32 LOW_HIGH ones, and
        # 4 go into one PSUM bank so a single wide ACT exp drains them.
        pew = tc.alloc_tile_pool(name="ew", bufs=2)
        pbs = tc.alloc_tile_pool(name="bstream", bufs=2)
        ppte = tc.alloc_tile_pool(name="psum_ew", bufs=2, space=PSUM)
        ppnd = tc.alloc_tile_pool(name="psum_nd", bufs=2, space=PSUM)
        BF16 = mybir.dt.bfloat16

        yt2T = [ptm.tile([P, T], F32R, tag=f"tm{dj}", name=f"yt2T{dj}")
                for dj in range(ND)]

        def produce_ewc(tc_i):
            wrow = pbs.tile([P, T], F32, tag="wrow")
            nc.sync.dma_start(
                out=wrow, in_=ap["w"][tc_i * P:(tc_i + 1) * P, :])
            wrow_bf = pbs.tile([P, T], BF16, tag="wrow_bf")
            nc.gpsimd.tensor_copy(out=wrow_bf, in_=wrow)
            ewc4 = [pew.tile([P, 4 * P], F32R, tag=f"ewc4_{g}",
                             name=f"ewc4_{g}") for g in range(NT // 4)]
            for g in range(NT // 4):
                pt4 = ppte.tile([P, 4 * P], BF16, tag="tpsum_ew")
                for k in range(4):
                    si = 4 * g + k
                    nc.tensor.transpose(
                        pt4[:, k * P:(k + 1) * P],
                        wrow_bf[:, si * P:(si + 1) * P], identity_bf)
                nc.scalar.activation(out=ewc4[g], in_=pt4, func=AF.Exp)
            return ewc4

        def consume_mms(tc_i, ewc4):
            pn = ppnd.tile([P, D], F32, tag="num")
            pd = ppnd.tile([P, D], F32, tag="den")
            for si in range(NT):
                nc.tensor.matmul(pn, ewc4[si // 4][:, (si % 4) * P:
                                                   (si % 4 + 1) * P],
                                 u_sb[si],
                                 start=(si == 0), stop=(si == NT - 1))
            for si in range(NT):
                nc.tensor.matmul(pd, ewc4[si // 4][:, (si % 4) * P:
                                                   (si % 4 + 1) * P],
                                 e_sb[si],
                                 start=(si == 0), stop=(si == NT - 1))
            rec = pbs.tile([P, D], F32, tag="rec")
            nc.vector.reciprocal_approx_fast(out=rec, in_=pd)
            ytt = pbs.tile([P, D], F32, tag="ytt")
            nc.vector.tensor_mul(out=ytt, in0=pn, in1=rec)
            yt2 = pbs.tile([P, D], F32, tag=f"yt2_{tc_i % 2}")
            nc.vector.tensor_mul(out=yt2, in0=ytt, in1=g_sb[tc_i])
            return yt2

        def emit_yt2T(tc_i, yt2):
            for dj in range(ND):
                _transpose_tile(nc, ppt, yt2T[dj], tc_i * P,
                                yt2[:, dj * P:(dj + 1) * P], identity)

        # two-level skew: EW production leads by one chunk; the yt2
        # transposes trail by one chunk so they never stall the PE on the
        # reciprocal/gate DVE chain.
        prev_ew = produce_ewc(0)
        prev_yt = None
        for tc_i in range(NT):
            nxt = produce_ewc(tc_i + 1) if tc_i + 1 < NT else None
            yt2 = consume_mms(tc_i, prev_ew)
            if prev_yt is not None:
                emit_yt2T(tc_i - 1, prev_yt)
            prev_ew, prev_yt = nxt, yt2
        emit_yt2T(NT - 1, prev_yt)
        ppnd.release()
        ppte.release()
        pbs.release()
        pew.release()
        pg.release()
        pu.release()
        pe.release()

        # ------ Stage C: attn-out + residual + LN2 + h2T ---------------
        pout = tc.alloc_tile_pool(name="rows_out", bufs=1)
        pwo = tc.alloc_tile_pool(name="wo", bufs=1)
        ps_c = tc.alloc_tile_pool(name="stream_c", bufs=2)
        ppm2 = tc.alloc_tile_pool(name="psum_mm2", bufs=2, space=PSUM)

        wo_sb = _load_rows(nc, pwo, ap["Wo"], ND, "wo", D, dtype=F32R)
        bo_bc = _bc(nc, pwo, ap["bo"], "bo_bc")
        g2_bc = _bc(nc, pwo, ap["ln2_g"], "g2_bc")
        b2g_bc = _bc(nc, pwo, ap["ln2_b"], "b2g_bc")

        out_sb = []
        for j in range(NT):
            pa_t = ppm2.tile([P, D], F32, tag="kv_psum")
            for dj in range(ND):
                nc.tensor.matmul(
                    pa_t, yt2T[dj][:, j * P:(j + 1) * P], wo_sb[dj],
                    start=(dj == 0), stop=(dj == ND - 1))
            x_tile = ps_c.tile([P, D], F32, tag="x")
            nc.sync.dma_start(out=x_tile, in_=ap["x"][j * P:(j + 1) * P, :])
            ot = pout.tile([P, D], F32, tag=f"o{j}")
            nc.vector.tensor_add(out=ot, in0=pa_t, in1=bo_bc)
            nc.vector.tensor_add(out=ot, in0=ot, in1=x_tile)
            out_sb.append(ot)

        mvall2 = pc.tile([P, 2 * NT], F32, name="mvall2")
        h2T = [ptm.tile([P, T], F32R, tag=f"tm{dj}", name=f"h2T{dj}")
               for dj in range(ND)]
        GRP = 4
        for g0 in range(0, NT, GRP):
            for j in range(g0, g0 + GRP):
                _ln_stats(nc, ps_c, out_sb[j], mvall2, j)
            rstd2 = _ln_rstd_batch(nc, ps_c, mvall2, GRP, eps_tile,
                                   f"r2_{g0}", col0=g0)
            for j in range(g0, g0 + GRP):
                h2_tile = _ln_apply(nc, ps_c, out_sb[j], mvall2, j, rstd2,
                                    g2_bc, b2g_bc, gp_apply=True,
                                    rcol=j - g0)
                for dj in range(ND):
                    _transpose_tile(nc, ppt, h2T[dj], j * P,
                                    h2_tile[:, dj * P:(dj + 1) * P], identity)
        ps_c.release()
        pwo.release()

        # ---------------- Stage D: FFN ---------------------------------
        pfw = tc.alloc_tile_pool(name="ffnw", bufs=1)
        pg1 = tc.alloc_tile_pool(name="g1", bufs=1)
        pds = tc.alloc_tile_pool(name="dstream", bufs=2)
        ppg = tc.alloc_tile_pool(name="psum_g", bufs=2, space=PSUM)

        w1_sb = _load_rows(nc, pfw, ap["W1"], ND, "w1_", H, dtype=F32R)
        w2_sb = _load_rows(nc, pfw, ap["W2"], NH, "w2_", D, dtype=F32R)
        b1_sb = _part_bias(nc, pfw, ap["b1"], NH, "b1_sb")
        b2_bc = _bc(nc, pfw, ap["b2"], "b2_bc")

        SW = 512                         # FFN strip width (moving dim)
        for ts2 in range(T // SW):       # 4 strips
            g1t = [pg1.tile([P, SW], F32R, tag=f"g1_{hk}", name=f"g1t{hk}")
                   for hk in range(NH)]
            for hk in range(NH):
                pg_t = ppg.tile([P, SW], F32, tag="g1psum")
                for dj in range(ND):
                    nc.tensor.matmul(
                        pg_t,
                        w1_sb[dj][:, hk * P:(hk + 1) * P],
                        h2T[dj][:, ts2 * SW:(ts2 + 1) * SW],
                        start=(dj == 0), stop=(dj == ND - 1))
                nc.scalar.activation(out=g1t[hk], in_=pg_t, func=AF.Gelu,
                                     bias=b1_sb[:, hk:hk + 1])
            for v in range(SW // P):     # 4 row-tiles per strip
                j = ts2 * (SW // P) + v
                pa2 = ppm2.tile([P, D], F32, tag="kv_psum")
                for hk in range(NH):
                    nc.tensor.matmul(
                        pa2, g1t[hk][:, v * P:(v + 1) * P], w2_sb[hk],
                        start=(hk == 0), stop=(hk == NH - 1))
                t1 = pds.tile([P, D], F32, tag="t1")
                nc.vector.tensor_add(out=t1, in0=pa2, in1=b2_bc)
                t2 = pds.tile([P, D], F32, tag="t2")
                nc.scalar.activation(out=t2, in_=t1, func=AF.Gelu)
                fin = pds.tile([P, D], F32, tag="fin")
                nc.vector.tensor_add(out=fin, in0=t2, in1=out_sb[j])
                nc.sync.dma_start(out=out_d[j * P:(j + 1) * P, :], in_=fin)

        for p in (ppg, pds, pg1, pfw, ppm2, ps_c, pout, ptm, pc, ppt):
            if not p._released:
                p.release()

    nc.compile()
    return nc


_CACHE = {}


def _get_programs():
    if "p1" not in _CACHE:
        _CACHE["p1"] = _build_phase1()
        _CACHE["p2"] = _build_phase2()
    return _CACHE["p1"], _CACHE["p2"]


def kernel(**inputs):
    np32 = {k: np.ascontiguousarray(np.asarray(v, dtype=np.float32))
            for k, v in inputs.items()}
    x = np32["x"]                      # [B, T, D]
    p1, p2 = _get_programs()
    LAST_RESULTS.clear()

    # Phase 1: seq-sharded batch-max of K
    in_maps1 = []
    for c in range(NCORES):
        xs = np.ascontiguousarray(
            x[:, c * TS:(c + 1) * TS, :].reshape(B * TS, D))
        in_maps1.append({
            "xs": xs,
            "ln1_g": np32["ln1_g"], "ln1_b": np32["ln1_b"],
            "Wk": np32["Wk"],
        })
    res1 = bass_utils.run_bass_kernel_spmd(p1, in_maps1,
                                           core_ids=list(range(NCORES)),
                                           trace=TRACE)
    LAST_RESULTS.append(res1)
    m0 = np.concatenate([res1.results[c]["M0"] for c in range(NCORES)], axis=0)

    # Phase 2: batch-sharded full block
    names = ["ln1_g", "ln1_b", "Wk", "Wv", "bv", "Wq", "bq", "Wo", "bo",
             "ln2_g", "ln2_b", "W1", "b1", "W2", "b2", "w"]
    shared = {n: np32[n] for n in names}
    in_maps2 = []
    for b in range(NCORES):
        m = {"x": np.ascontiguousarray(x[b]), "M0": m0}
        m.update(shared)
        in_maps2.append(m)
    res2 = bass_utils.run_bass_kernel_spmd(p2, in_maps2,
                                           core_ids=list(range(NCORES)),
                                           trace=TRACE)
    LAST_RESULTS.append(res2)
    out = np.stack([res2.results[b]["out"] for b in range(NCORES)], axis=0)
    return out

